# revision 37
# baseline (speedup 1.0000x reference)
"""Trainium2 Bass kernel for nn_LowRankOrthogonalMixer (B=8, N=4096, F=512, R=16).

Math: the reference builds per-batch skew matrices G = gate*(A - A^T) with
A = (left*coeff) @ right^T, combines them into
Omega = 0.5*(G+L) + comm/12*(LG-GL), applies the Cayley transform
T = (I-0.5*Omega)^{-1}(I+0.5*Omega), and mixes: out = x @ T.

Key structure exploited: Omega = P @ S @ Q^T with P,Q in [F,64] (rank<=64:
column/row spaces of both skews and their commutator live in
span[left,right,left_local,right_local]). With W = P*(0.5*S) (so
0.5*Omega = W Q^T) and C = I64 - Q^T W, Woodbury collapses the Cayley
transform EXACTLY to
    T = I + 2 W C^{-1} Q^T   =>   out = x + (x @ W) @ (2 C^{-1}) @ Q^T.
C^{-1} comes from 7 Newton-Schulz iterations (V0 = 0.22*C^T, near-optimal
for the measured sigma(C) range [0.55, 2.9]).

Schedule (DMA-roofline oriented; per NeuronCore, data-parallel over B):
- the output is written to HBM as bf16 (quantization ~2e-3 relative, well
  inside the 2e-2 gate) and cast back to fp32 on the host: HBM traffic is
  8 MB x-in + 4 MB out instead of 8+8 (roofline ~35us wire time).
- DMA moves in 2-tile (512 KB in / 256 KB out) units: dma_start costs
  ~650 ns of issuing-engine time regardless of size. All 16 x-in DMAs are
  issued up-front from Sync into dedicated SBUF (no recycling -> no
  anti-deps -> the queue streams 8 MB at line rate); out DMAs alternate
  between Sync and GpSimd.
- phase 0 (C, Newton-Schulz, ZT = 2 C^{-1} Q^T) is emitted at scheduler
  high priority with its latency chain on an otherwise-EMPTY DVE: the
  streaming copies live on ACT, and the DVE residual adds are all gated on
  ztm anyway, so nothing queues ahead of the chain hops in the strict
  in-order DVE FIFO. ztm lands ~15-20us instead of ~37 (the v1 gating).
- phase 1 streams x in 128-row tiles, groups of 4: 4x 128x128 fp32
  PE-transposes per tile, ACT copies the PSUM result to a bf16 xt4
  staging tile, mm1 = W^T x^T in bf16 over 4 F-chunks at N=512,
  mm2 = u @ ZT at N=512 in f32r, DVE adds the fp32 residual from PSUM and
  writes bf16 output pairs which DMA straight out.
- keep-warm: transpose-mode doesn't register as PE activity for the HAM
  clock gate, so a dummy bf16 matmul is sprinkled every other tile
  through the front (plus a ~2.6us warm-up burst at the start).

Sharding: data-parallel over batch B=8 -> one batch item per NeuronCore.
"""

import numpy as np

import concourse.bass as bass
import concourse.bacc as bacc
import concourse.tile as tile
from concourse import mybir
from concourse.bass_utils import run_bass_kernel_spmd

B, N, F, R = 8, 4096, 512, 16
NTILES = N // 128
GT = 4                      # tiles per mm1 group
NPAIR = NTILES // 2         # 2-tile DMA units
ALPHA = 0.22  # Newton-Schulz init scale: V0 = ALPHA * C^T. Near-optimal for the
# measured sigma(C) range [0.55, 2.9]: |1 - a*s^2| <= max(1-0.22*0.30, 0.22*8.41-1)
# = 0.934 -> 7 iterations reach ~1.5e-4; divergence guard a*s_max^2 = 1.85 < 2.
NS_ITERS = 7
WARMUP_MMS = 24

# packed setup tensor column layout
_C_SMALLS = 0        # [128, 512]: rows 0:64 Q^T sources, 64:128 P^T sources
_C_IDENT = 512       # [128, 128] identity
_C_E0 = 640          # [64, 64] +-1/24 commutator mask (rows 64:128 zero)
_C_BASE = 704        # qp_base column
_C_GATE = 705        # qp_gate column
_C_SIGN = 706        # qp_sign column
_C_CVEC = 707        # comm_scale broadcast column (rows 0:64)
_C_II = 708          # [64, 128] = [I64 | I64] (rows 64:128 zero)
SETUP_COLS = 836

_CACHE = {}


def build_bass():
    # Bacc (not plain Bass): its compile() runs move_matmul_waits_to_ldweights
    # + generate_event_semaphores, required because TRN2 instructions support
    # at most one semaphore wait each.
    nc = bacc.Bacc(trn_type="TRN2", target_bir_lowering=False)
    dt = mybir.dt.float32
    bf16 = mybir.dt.bfloat16
    f32r = mybir.dt.float32r

    x_d = nc.dram_tensor("x", [N, F], dt, kind="ExternalInput")
    setup_d = nc.dram_tensor("setup", [128, SETUP_COLS], dt, kind="ExternalInput")
    out_d = nc.dram_tensor("out", [N, F], bf16, kind="ExternalOutput")

    with tile.TileContext(nc) as tc:
        with (
            tc.tile_pool(name="const", bufs=1) as const,
            tc.tile_pool(name="small", bufs=2) as small,
            # all 16 x-in DMAs land in distinct bufs: no anti-deps, the DMA
            # queue streams the full 8 MB back-to-back at line rate
            tc.tile_pool(name="xs", bufs=NPAIR) as xs,
            tc.tile_pool(name="xts", bufs=3) as xts,
            tc.tile_pool(name="us", bufs=4) as us,
            tc.tile_pool(name="outs", bufs=6) as outs,
            tc.tile_pool(name="ps_sm", bufs=2, space="PSUM") as ps_sm,
            tc.tile_pool(name="ps_str", bufs=2, space="PSUM") as ps_str,
            tc.tile_pool(name="ps_u", bufs=1, space="PSUM") as ps_u_pool,
            tc.tile_pool(name="ps_o", bufs=2, space="PSUM") as ps_o_pool,
            tc.tile_pool(name="ps_fill", bufs=1, space="PSUM") as ps_fill_pool,
        ):
            ps_once = ps_sm
            # PE warm-up: ~2.6us of dummy matmuls bridge the gap until the
            # first x tiles arrive (~10us: ring-init + first 512KB DMA) and
            # open the HAM clock gate (K=8/8, 2.4 GHz) before real work.
            warm_src = const.tile([128, 128], bf16)
            nc.vector.memset(warm_src, 0.0)
            ps_warm = ps_fill_pool.tile([128, 128], dt)
            for _ in range(WARMUP_MMS):
                nc.tensor.matmul(ps_warm, warm_src, warm_src,
                                 start=True, stop=True)

            # ---- setup DMAs first on Sync, then ALL x-in DMAs up-front ----
            setup = const.tile([128, SETUP_COLS], dt)
            nc.sync.dma_start(setup, setup_d[:, :])
            setup_p = const.tile([64, SETUP_COLS], dt)
            nc.sync.dma_start(setup_p, setup_d[64:128, :])

            # x as 2-tile units: xi2[k][:, 512*j:512*(j+1)] = x rows
            # 256k+128j .. 256k+128j+128
            x_pair = x_d[:, :].rearrange("(k j p) f -> k p j f", j=2, p=128)
            o_pair = out_d[:, :].rearrange("(k j p) f -> k p j f", j=2, p=128)
            xi2s = []
            for k in range(NPAIR):
                xi2 = xs.tile([128, 1024], dt, tag="xi2")
                nc.sync.dma_start(
                    xi2[:, :].rearrange("p (j f) -> p j f", j=2), x_pair[k]
                )
                xi2s.append(xi2)

            # ---- phase 0 (high priority): W, Q^T, C, C^{-1}, ZT = 2 C^{-1} Q^T ----
            # latency chain on DVE (empty until ztm exists: the residual adds
            # are gated on it); one-off big copies on ACT.
            hp = tc.high_priority()
            hp.__enter__()
            smalls = setup[:, _C_SMALLS:_C_SMALLS + 512]
            ident = setup[:, _C_IDENT:_C_IDENT + 128]
            i64 = setup[0:64, _C_IDENT:_C_IDENT + 64]
            e0 = setup[0:64, _C_E0:_C_E0 + 64]
            base_v = setup[:, _C_BASE:_C_BASE + 1]
            gate_v = setup[:, _C_GATE:_C_GATE + 1]
            sign_v = setup[:, _C_SIGN:_C_SIGN + 1]
            cv = setup[0:64, _C_CVEC:_C_CVEC + 1]

            # qp rows 0:64 = Q^T, rows 64:128 = P^T
            scale = small.tile([128, 1], dt, tag="scale")
            nc.vector.tensor_mul(scale, base_v, gate_v)
            scale2 = small.tile([128, 1], dt, tag="scale2")
            nc.vector.tensor_mul(scale2, scale, sign_v)
            qp = const.tile([128, F], dt)
            nc.vector.tensor_scalar_mul(qp, in0=smalls, scalar1=scale2)
            qt_ap = qp[0:64, :]
            # P^T at base partition 0 for the W^T matmul
            scale_p = small.tile([64, 1], dt, tag="scale_p")
            nc.vector.tensor_mul(
                scale_p,
                setup_p[:, _C_BASE:_C_BASE + 1],
                setup_p[:, _C_GATE:_C_GATE + 1],
            )
            scale_p2 = small.tile([64, 1], dt, tag="scale_p2")
            nc.vector.tensor_mul(scale_p2, scale_p, setup_p[:, _C_SIGN:_C_SIGN + 1])
            pt0 = const.tile([64, F], f32r)
            nc.vector.tensor_scalar_mul(
                pt0, in0=setup_p[:, _C_SMALLS:_C_SMALLS + 512], scalar1=scale_p2
            )

            # naturals: qpn block c (cols 128c..128c+128) = (qp[:,128c:128c+128])^T
            ps_qpn = ps_once.tile([128, 512], dt, tag="ns_ps")
            for c in range(4):
                nc.tensor.transpose(
                    ps_qpn[:, 128 * c : 128 * (c + 1)],
                    qp[:, 128 * c : 128 * (c + 1)],
                    ident,
                )
            qpn = const.tile([128, 512], f32r)
            nc.scalar.copy(qpn, ps_qpn)

            # G1 = Q^T P and G1T = P^T Q side-by-side in ONE PSUM bank so a
            # single DVE cast extracts both (fewer chain hops)
            ps_gg = ps_sm.tile([64, 128], dt, tag="ns_ps")
            for c in range(4):
                qch = qpn[:, 128 * c : 128 * c + 64]
                pch = qpn[:, 128 * c + 64 : 128 * (c + 1)]
                nc.tensor.matmul(ps_gg[:, 0:64], qch, pch, start=(c == 0), stop=(c == 3))
            for c in range(4):
                qch = qpn[:, 128 * c : 128 * c + 64]
                pch = qpn[:, 128 * c + 64 : 128 * (c + 1)]
                nc.tensor.matmul(ps_gg[:, 64:128], pch, qch, start=(c == 0), stop=(c == 3))
            gg = small.tile([64, 128], f32r, tag="gg")
            nc.vector.tensor_copy(gg, ps_gg)
            g1 = gg[:, 0:64]
            g1t = gg[:, 64:128]

            # S_half = 0.25*I + comm * (e0 ⊙ G1)   (e0 carries the ±1/24 pattern)
            e0c = small.tile([64, 64], dt, tag="e0c")
            nc.vector.tensor_scalar_mul(e0c, in0=e0, scalar1=cv)
            s_half = small.tile([64, 64], f32r, tag="s_half")
            nc.vector.tensor_mul(s_half, e0c, g1)
            i4 = small.tile([64, 64], dt, tag="i4")
            nc.scalar.mul(i4, i64, 0.25)
            nc.vector.tensor_add(s_half, s_half, i4)

            # cc = [C | C^T] = [I|I] - [G1 S_half | (G1 S_half)^T], one bank +
            # one DVE sub
            ii = setup[0:64, _C_II:_C_II + 128]
            ps_cc = ps_sm.tile([64, 128], dt, tag="ns_ps")
            nc.tensor.matmul(ps_cc[:, 0:64], g1t, s_half, start=True, stop=True)
            nc.tensor.matmul(ps_cc[:, 64:128], s_half, g1t, start=True, stop=True)
            cc = small.tile([64, 128], f32r, tag="cc")
            nc.vector.tensor_sub(cc, ii, ps_cc)
            cmat = cc[:, 0:64]     # C
            ctm = cc[:, 64:128]    # C^T

            # W^T = S_half^T @ P^T  [64, F]; then W natural in 4 chunks [128, 64]
            # (off the NS critical path; ACT does the copies; W in bf16 for mm1)
            ps_wt = ps_once.tile([128, 512], dt, tag="ns_ps")
            nc.tensor.matmul(ps_wt[0:64, :], s_half, pt0, start=True, stop=True)
            wtm = const.tile([64, 512], dt)
            nc.scalar.copy(wtm, ps_wt[0:64, :])
            ps_w = ps_once.tile([128, 512], dt, tag="ns_ps")
            for c in range(4):
                nc.tensor.transpose(
                    ps_w[:, 64 * c : 64 * (c + 1)],
                    wtm[:, 128 * c : 128 * (c + 1)],
                    i64,
                )
            wm = const.tile([128, 256], bf16)
            nc.scalar.copy(wm, ps_w[:, 0:256])

            # Newton-Schulz for V = C^{-1}. vv = [V^T | V]: both iterates live
            # side-by-side so each iteration is mm -> sub -> 2x mm (one bank)
            # -> ONE cast. (V error after 7 iters ~1.5e-4, far below the bf16
            # rounding of ztm, so everything stays f32r.)
            i2 = small.tile([64, 64], f32r, tag="i2")
            nc.scalar.mul(i2, i64, 2.0)
            vv = small.tile([64, 128], f32r, tag="vv")
            nc.vector.tensor_scalar_mul(vv, in0=cc, scalar1=ALPHA)  # [a*C | a*C^T] = [vt0 | v0]
            for it in range(NS_ITERS):
                ps_t1 = ps_sm.tile([64, 128], dt, tag="ns_ps")
                nc.tensor.matmul(ps_t1[:, 0:64], ctm, vv[:, 64:128],
                                 start=True, stop=True)              # C V
                t2 = small.tile([64, 64], f32r, tag="t2")
                nc.vector.tensor_sub(t2, i2, ps_t1[:, 0:64])         # 2I - CV
                ps_vv = ps_sm.tile([64, 128], dt, tag="ns_ps")
                nc.tensor.matmul(ps_vv[:, 0:64], t2, vv[:, 0:64],
                                 start=True, stop=True)              # t2^T V^T = vt'
                nc.tensor.matmul(ps_vv[:, 64:128], vv[:, 0:64], t2,
                                 start=True, stop=True)              # V t2 = v'
                vv = small.tile([64, 128], f32r, tag="vv")
                nc.vector.tensor_copy(vv, ps_vv)

            # ZT = 2 * V @ Q^T  [64, F] (bf16 for the FWL-accelerated mm2)
            ps_zt = ps_once.tile([128, 512], dt, tag="ns_ps")
            nc.tensor.matmul(ps_zt[0:64, :], vv[:, 0:64].bitcast(dt), qt_ap,
                             start=True, stop=True)
            ztm = const.tile([64, 512], bf16)
            nc.vector.tensor_scalar_mul(ztm, in0=ps_zt[0:64, :], scalar1=2.0)
            hp.__exit__(None, None, None)

            # keep-warm matmuls spread through the phase-0 window (PE is
            # otherwise idle there and the HAM clock gate would re-throttle)
            for i in range(14):
                with tc.tile_wait_until(0.0100 + 0.0005 * i):
                    nc.tensor.matmul(ps_warm, warm_src, warm_src,
                                     start=True, stop=True)

            # ---- phase 1: stream x tiles in groups of 4 ----
            # The whole streaming pipeline is time-gated past the Newton-Schulz
            # window: otherwise the static scheduler packs transpose bursts
            # into every NS dependency gap and stretches the phase-0 latency
            # chain (which gates mm2 and all output DMA) by 2x.
            gate = tc.tile_wait_until(0.016)
            gate.__enter__()
            for g in range(NTILES // GT):
                # xt4 layout [128, (c t n)]: chunk c of all GT tiles adjacent so
                # mm1's rhs for chunk c is the contiguous slice [:, 512c:512c+512].
                # bf16 staging (converted in the ACT copy): only the
                # ~17%-magnitude correction term sees the rounding.
                xt4 = xts.tile([128, GT * 512], bf16, tag="xt4")
                xt4_v = xt4[:, :].rearrange("p (c t n) -> p c t n", c=4, t=GT)
                for t in range(GT):
                    xi = xi2s[2 * g + t // 2][:, 512 * (t % 2) : 512 * (t % 2) + 512]
                    ps_xt = ps_str.tile([128, 512], dt, tag="ps_xt")
                    for c in range(4):
                        nc.tensor.transpose(
                            ps_xt[:, 128 * c : 128 * (c + 1)],
                            xi[:, 128 * c : 128 * (c + 1)],
                            ident,
                        )
                    nc.scalar.copy(
                        xt4_v[:, :, t, :],
                        ps_xt[:, :].rearrange("p (c n) -> p c n", c=4),
                    )
                ps_u4 = ps_u_pool.tile([64, 512], dt, tag="ps_u2")
                for c in range(4):
                    nc.tensor.matmul(
                        ps_u4,
                        wm[:, 64 * c : 64 * (c + 1)],
                        xt4[:, 512 * c : 512 * (c + 1)],
                        start=(c == 0),
                        stop=(c == 3),
                    )
                u4 = us.tile([64, 512], bf16, tag="u2")
                nc.vector.tensor_copy(u4, ps_u4)
                for half in range(2):
                    ob = outs.tile([128, 1024], bf16, tag="ob")
                    for j in range(2):
                        t = 2 * half + j
                        xi = xi2s[2 * g + t // 2][:, 512 * (t % 2) : 512 * (t % 2) + 512]
                        ps_o = ps_o_pool.tile([128, 512], dt, tag="ps_o")
                        nc.tensor.matmul(
                            ps_o,
                            u4[:, 128 * t : 128 * (t + 1)],
                            ztm,
                            start=True,
                            stop=True,
                        )
                        nc.vector.tensor_add(ob[:, 512 * j : 512 * (j + 1)], xi, ps_o)
                    eng = nc.sync if half == 0 else nc.gpsimd
                    eng.dma_start(
                        o_pair[2 * g + half],
                        ob[:, :].rearrange("p (j f) -> p j f", j=2),
                    )
            gate.__exit__(None, None, None)

    return nc


def make_setup(coeff_b, gate_b, coeff_l_b, gate_l_b, comm_b,
               left, right, left_local, right_local):
    """Pack all small inputs for one batch item into one [128, 708] tensor.
    Pure marshalling: transposes/replication of raw inputs plus constants."""
    f32 = np.float32
    s = np.zeros((128, SETUP_COLS), f32)
    s[:, 0:512] = np.concatenate(
        [right.T, left.T, right_local.T, left_local.T,
         left.T, right.T, left_local.T, right_local.T], axis=0
    )
    s[:, _C_IDENT:_C_IDENT + 128] = np.eye(128, dtype=f32)
    s[0:32, _C_E0 + 32:_C_E0 + 64] = -1.0 / 24.0
    s[32:64, _C_E0:_C_E0 + 32] = 1.0 / 24.0
    ones16 = np.ones(16, f32)
    s[:, _C_BASE] = np.concatenate(
        [ones16, coeff_b, ones16, coeff_l_b, coeff_b, ones16, coeff_l_b, ones16]
    )
    s[:, _C_GATE] = np.concatenate(
        [np.ones(64, f32), np.full(32, gate_b, f32), np.full(32, gate_l_b, f32)]
    )
    s[:, _C_SIGN] = np.concatenate(
        [np.ones(80, f32), -np.ones(16, f32), np.ones(16, f32), -np.ones(16, f32)]
    )
    s[0:64, _C_CVEC] = comm_b
    s[0:64, _C_II:_C_II + 64] = np.eye(64, dtype=f32)
    s[0:64, _C_II + 64:_C_II + 128] = np.eye(64, dtype=f32)
    return s


def make_in_maps(x, coeff, gate, coeff_local, gate_local, comm_scale,
                 left, right, left_local, right_local):
    in_maps = []
    for b in range(x.shape[0]):
        in_maps.append({
            "x": np.ascontiguousarray(x[b]).astype(np.float32),
            "setup": make_setup(coeff[b], gate[b], coeff_local[b], gate_local[b],
                                comm_scale[b], left, right, left_local, right_local),
        })
    return in_maps


def kernel(x, coeff, gate, coeff_local, gate_local, comm_scale,
           left, right, left_local, right_local, _trace=False):
    if "nc" not in _CACHE:
        nc = build_bass()
        nc.finalize()  # Bacc.finalize: compile passes + freeze
        _CACHE["nc"] = nc
    nc = _CACHE["nc"]
    in_maps = make_in_maps(x, coeff, gate, coeff_local, gate_local, comm_scale,
                           left, right, left_local, right_local)
    res = run_bass_kernel_spmd(nc, in_maps, core_ids=list(range(8)), trace=_trace)
    out = np.stack([r["out"] for r in res.results], axis=0)
    if _trace:
        _CACHE["last_results"] = res
    return out.astype(x.dtype)


# revision 40
# speedup vs baseline: 1.0196x; 1.0196x over previous
"""Trainium2 Bass kernel for nn_LowRankOrthogonalMixer (B=8, N=4096, F=512, R=16).

Math: the reference builds per-batch skew matrices G = gate*(A - A^T) with
A = (left*coeff) @ right^T, combines them into
Omega = 0.5*(G+L) + comm/12*(LG-GL), applies the Cayley transform
T = (I-0.5*Omega)^{-1}(I+0.5*Omega), and mixes: out = x @ T.

Key structure exploited: Omega = P @ S @ Q^T with P,Q in [F,64] (rank<=64:
column/row spaces of both skews and their commutator live in
span[left,right,left_local,right_local]). With W = P*(0.5*S) (so
0.5*Omega = W Q^T) and C = I64 - Q^T W, Woodbury collapses the Cayley
transform EXACTLY to
    T = I + 2 W C^{-1} Q^T   =>   out = x + (x @ W) @ (2 C^{-1}) @ Q^T.
C^{-1} comes from 7 Newton-Schulz iterations (V0 = 0.22*C^T, near-optimal
for the measured sigma(C) range [0.55, 2.9]).

Schedule (DMA-roofline oriented; per NeuronCore, data-parallel over B):
- the output is written to HBM as bf16 (quantization ~2e-3 relative, well
  inside the 2e-2 gate) and cast back to fp32 on the host: HBM traffic is
  8 MB x-in + 4 MB out instead of 8+8 (roofline ~35us wire time).
- DMA moves in 2-tile (512 KB in / 256 KB out) units: dma_start costs
  ~650 ns of issuing-engine time regardless of size. All 16 x-in DMAs are
  issued up-front from Sync into dedicated SBUF (no recycling -> no
  anti-deps -> the queue streams 8 MB at line rate); out DMAs alternate
  between Sync and GpSimd.
- phase 0 (C, Newton-Schulz, ZT = 2 C^{-1} Q^T) is emitted at scheduler
  high priority with its latency chain on an otherwise-EMPTY DVE: the
  streaming copies live on ACT, and the DVE residual adds are all gated on
  ztm anyway, so nothing queues ahead of the chain hops in the strict
  in-order DVE FIFO. ztm lands ~15-20us instead of ~37 (the v1 gating).
- phase 1 streams x in 128-row tiles, groups of 4: 4x 128x128 fp32
  PE-transposes per tile, ACT copies the PSUM result to a bf16 xt4
  staging tile, mm1 = W^T x^T in bf16 over 4 F-chunks at N=512,
  mm2 = u @ ZT at N=512 in f32r, DVE adds the fp32 residual from PSUM and
  writes bf16 output pairs which DMA straight out.
- keep-warm: transpose-mode doesn't register as PE activity for the HAM
  clock gate, so a dummy bf16 matmul is sprinkled every other tile
  through the front (plus a ~2.6us warm-up burst at the start).

Sharding: data-parallel over batch B=8 -> one batch item per NeuronCore.
"""

import numpy as np

import concourse.bass as bass
import concourse.bacc as bacc
import concourse.tile as tile
from concourse import mybir
from concourse.bass_utils import run_bass_kernel_spmd

B, N, F, R = 8, 4096, 512, 16
NTILES = N // 128
GT = 4                      # tiles per mm1 group
NPAIR = NTILES // 2         # 2-tile DMA units
ALPHA = 0.22  # Newton-Schulz init scale: V0 = ALPHA * C^T. Near-optimal for the
# measured sigma(C) range [0.55, 2.9]: |1 - a*s^2| <= max(1-0.22*0.30, 0.22*8.41-1)
# = 0.934 -> 7 iterations reach ~1.5e-4; divergence guard a*s_max^2 = 1.85 < 2.
NS_ITERS = 7
WARMUP_MMS = 24

# packed setup tensor column layout
_C_SMALLS = 0        # [128, 512]: rows 0:64 Q^T sources, 64:128 P^T sources
_C_IDENT = 512       # [128, 128] identity
_C_E0 = 640          # [64, 64] +-1/24 commutator mask (rows 64:128 zero)
_C_BASE = 704        # qp_base column
_C_GATE = 705        # qp_gate column
_C_SIGN = 706        # qp_sign column
_C_CVEC = 707        # comm_scale broadcast column (rows 0:64)
_C_II = 708          # [64, 128] = [I64 | I64] (rows 64:128 zero)
SETUP_COLS = 836

_CACHE = {}


def build_bass():
    # Bacc (not plain Bass): its compile() runs move_matmul_waits_to_ldweights
    # + generate_event_semaphores, required because TRN2 instructions support
    # at most one semaphore wait each.
    nc = bacc.Bacc(trn_type="TRN2", target_bir_lowering=False)
    dt = mybir.dt.float32
    bf16 = mybir.dt.bfloat16
    f32r = mybir.dt.float32r

    x_d = nc.dram_tensor("x", [N, F], dt, kind="ExternalInput")
    setup_d = nc.dram_tensor("setup", [128, SETUP_COLS], dt, kind="ExternalInput")
    out_d = nc.dram_tensor("out", [N, F], bf16, kind="ExternalOutput")

    with tile.TileContext(nc) as tc:
        with (
            tc.tile_pool(name="const", bufs=1) as const,
            tc.tile_pool(name="small", bufs=2) as small,
            # all 16 x-in DMAs land in distinct bufs: no anti-deps, the DMA
            # queue streams the full 8 MB back-to-back at line rate
            tc.tile_pool(name="xs", bufs=NPAIR) as xs,
            tc.tile_pool(name="xts", bufs=3) as xts,
            tc.tile_pool(name="us", bufs=4) as us,
            tc.tile_pool(name="outs", bufs=6) as outs,
            tc.tile_pool(name="ps_sm", bufs=2, space="PSUM") as ps_sm,
            tc.tile_pool(name="ps_str", bufs=2, space="PSUM") as ps_str,
            tc.tile_pool(name="ps_u", bufs=1, space="PSUM") as ps_u_pool,
            tc.tile_pool(name="ps_o", bufs=2, space="PSUM") as ps_o_pool,
            tc.tile_pool(name="ps_fill", bufs=1, space="PSUM") as ps_fill_pool,
        ):
            ps_once = ps_sm
            # PE warm-up: ~2.6us of dummy matmuls bridge the gap until the
            # first x tiles arrive (~10us: ring-init + first 512KB DMA) and
            # open the HAM clock gate (K=8/8, 2.4 GHz) before real work.
            warm_src = const.tile([128, 128], bf16)
            nc.vector.memset(warm_src, 0.0)
            ps_warm = ps_fill_pool.tile([128, 128], dt)
            for _ in range(WARMUP_MMS):
                nc.tensor.matmul(ps_warm, warm_src, warm_src,
                                 start=True, stop=True)

            # ---- setup DMAs first on Sync, then ALL x-in DMAs up-front ----
            setup = const.tile([128, SETUP_COLS], dt)
            nc.sync.dma_start(setup, setup_d[:, :])
            setup_p = const.tile([64, SETUP_COLS], dt)
            nc.sync.dma_start(setup_p, setup_d[64:128, :])

            # x as 2-tile units: xi2[k][:, 512*j:512*(j+1)] = x rows
            # 256k+128j .. 256k+128j+128
            x_pair = x_d[:, :].rearrange("(k j p) f -> k p j f", j=2, p=128)
            o_pair = out_d[:, :].rearrange("(k j p) f -> k p j f", j=2, p=128)
            xi2s = []
            for k in range(NPAIR):
                xi2 = xs.tile([128, 1024], dt, tag="xi2")
                nc.sync.dma_start(
                    xi2[:, :].rearrange("p (j f) -> p j f", j=2), x_pair[k]
                )
                xi2s.append(xi2)

            # ---- phase 0 (high priority): W, Q^T, C, C^{-1}, ZT = 2 C^{-1} Q^T ----
            # latency chain on DVE (empty until ztm exists: the residual adds
            # are gated on it); one-off big copies on ACT.
            hp = tc.high_priority()
            hp.__enter__()
            smalls = setup[:, _C_SMALLS:_C_SMALLS + 512]
            ident = setup[:, _C_IDENT:_C_IDENT + 128]
            i64 = setup[0:64, _C_IDENT:_C_IDENT + 64]
            e0 = setup[0:64, _C_E0:_C_E0 + 64]
            base_v = setup[:, _C_BASE:_C_BASE + 1]
            gate_v = setup[:, _C_GATE:_C_GATE + 1]
            sign_v = setup[:, _C_SIGN:_C_SIGN + 1]
            cv = setup[0:64, _C_CVEC:_C_CVEC + 1]

            # qp rows 0:64 = Q^T, rows 64:128 = P^T
            scale = small.tile([128, 1], dt, tag="scale")
            nc.vector.tensor_mul(scale, base_v, gate_v)
            scale2 = small.tile([128, 1], dt, tag="scale2")
            nc.vector.tensor_mul(scale2, scale, sign_v)
            qp = const.tile([128, F], dt)
            nc.vector.tensor_scalar_mul(qp, in0=smalls, scalar1=scale2)
            qt_ap = qp[0:64, :]
            # P^T at base partition 0 for the W^T matmul
            scale_p = small.tile([64, 1], dt, tag="scale_p")
            nc.vector.tensor_mul(
                scale_p,
                setup_p[:, _C_BASE:_C_BASE + 1],
                setup_p[:, _C_GATE:_C_GATE + 1],
            )
            scale_p2 = small.tile([64, 1], dt, tag="scale_p2")
            nc.vector.tensor_mul(scale_p2, scale_p, setup_p[:, _C_SIGN:_C_SIGN + 1])
            pt0 = const.tile([64, F], f32r)
            nc.vector.tensor_scalar_mul(
                pt0, in0=setup_p[:, _C_SMALLS:_C_SMALLS + 512], scalar1=scale_p2
            )

            # naturals: qpn block c (cols 128c..128c+128) = (qp[:,128c:128c+128])^T
            ps_qpn = ps_once.tile([128, 512], dt, tag="ns_ps")
            for c in range(4):
                nc.tensor.transpose(
                    ps_qpn[:, 128 * c : 128 * (c + 1)],
                    qp[:, 128 * c : 128 * (c + 1)],
                    ident,
                )
            qpn = const.tile([128, 512], f32r)
            nc.scalar.copy(qpn, ps_qpn)

            # G1 = Q^T P and G1T = P^T Q side-by-side in ONE PSUM bank so a
            # single DVE cast extracts both (fewer chain hops)
            ps_gg = ps_sm.tile([64, 128], dt, tag="ns_ps")
            for c in range(4):
                qch = qpn[:, 128 * c : 128 * c + 64]
                pch = qpn[:, 128 * c + 64 : 128 * (c + 1)]
                nc.tensor.matmul(ps_gg[:, 0:64], qch, pch, start=(c == 0), stop=(c == 3))
            for c in range(4):
                qch = qpn[:, 128 * c : 128 * c + 64]
                pch = qpn[:, 128 * c + 64 : 128 * (c + 1)]
                nc.tensor.matmul(ps_gg[:, 64:128], pch, qch, start=(c == 0), stop=(c == 3))
            gg = small.tile([64, 128], f32r, tag="gg")
            nc.vector.tensor_copy(gg, ps_gg)
            g1 = gg[:, 0:64]
            g1t = gg[:, 64:128]

            # S_half = 0.25*I + comm * (e0 ⊙ G1)   (e0 carries the ±1/24 pattern)
            e0c = small.tile([64, 64], dt, tag="e0c")
            nc.vector.tensor_scalar_mul(e0c, in0=e0, scalar1=cv)
            s_half = small.tile([64, 64], f32r, tag="s_half")
            nc.vector.tensor_mul(s_half, e0c, g1)
            i4 = small.tile([64, 64], dt, tag="i4")
            nc.scalar.mul(i4, i64, 0.25)
            nc.vector.tensor_add(s_half, s_half, i4)

            # cc = [C | C^T] = [I|I] - [G1 S_half | (G1 S_half)^T], one bank +
            # one DVE sub
            ii = setup[0:64, _C_II:_C_II + 128]
            ps_cc = ps_sm.tile([64, 128], dt, tag="ns_ps")
            nc.tensor.matmul(ps_cc[:, 0:64], g1t, s_half, start=True, stop=True)
            nc.tensor.matmul(ps_cc[:, 64:128], s_half, g1t, start=True, stop=True)
            cc = small.tile([64, 128], f32r, tag="cc")
            nc.vector.tensor_sub(cc, ii, ps_cc)
            cmat = cc[:, 0:64]     # C
            ctm = cc[:, 64:128]    # C^T

            # W^T = S_half^T @ P^T  [64, F]; then W natural in 4 chunks [128, 64]
            # (off the NS critical path; ACT does the copies; W in bf16 for mm1)
            ps_wt = ps_once.tile([128, 512], dt, tag="ns_ps")
            nc.tensor.matmul(ps_wt[0:64, :], s_half, pt0, start=True, stop=True)
            wtm = const.tile([64, 512], dt)
            nc.scalar.copy(wtm, ps_wt[0:64, :])
            ps_w = ps_once.tile([128, 512], dt, tag="ns_ps")
            for c in range(4):
                nc.tensor.transpose(
                    ps_w[:, 64 * c : 64 * (c + 1)],
                    wtm[:, 128 * c : 128 * (c + 1)],
                    i64,
                )
            wm = const.tile([128, 256], bf16)
            nc.scalar.copy(wm, ps_w[:, 0:256])

            # Newton-Schulz for V = C^{-1}. vv = [V^T | V]: both iterates live
            # side-by-side so each iteration is mm -> sub -> 2x mm (one bank)
            # -> ONE cast. (V error after 7 iters ~1.5e-4, far below the bf16
            # rounding of ztm, so everything stays f32r.)
            i2 = small.tile([64, 64], f32r, tag="i2")
            nc.scalar.mul(i2, i64, 2.0)
            vv = small.tile([64, 128], f32r, tag="vv")
            nc.vector.tensor_scalar_mul(vv, in0=cc, scalar1=ALPHA)  # [a*C | a*C^T] = [vt0 | v0]
            for it in range(NS_ITERS):
                ps_t1 = ps_sm.tile([64, 128], dt, tag="ns_ps")
                nc.tensor.matmul(ps_t1[:, 0:64], ctm, vv[:, 64:128],
                                 start=True, stop=True)              # C V
                t2 = small.tile([64, 64], f32r, tag="t2")
                nc.vector.tensor_sub(t2, i2, ps_t1[:, 0:64])         # 2I - CV
                ps_vv = ps_sm.tile([64, 128], dt, tag="ns_ps")
                nc.tensor.matmul(ps_vv[:, 0:64], t2, vv[:, 0:64],
                                 start=True, stop=True)              # t2^T V^T = vt'
                nc.tensor.matmul(ps_vv[:, 64:128], vv[:, 0:64], t2,
                                 start=True, stop=True)              # V t2 = v'
                vv = small.tile([64, 128], f32r, tag="vv")
                nc.vector.tensor_copy(vv, ps_vv)

            # ZT = 2 * V @ Q^T  [64, F] (bf16 for the FWL-accelerated mm2)
            ps_zt = ps_once.tile([128, 512], dt, tag="ns_ps")
            nc.tensor.matmul(ps_zt[0:64, :], vv[:, 0:64].bitcast(dt), qt_ap,
                             start=True, stop=True)
            ztm = const.tile([64, 512], bf16)
            nc.vector.tensor_scalar_mul(ztm, in0=ps_zt[0:64, :], scalar1=2.0)
            hp.__exit__(None, None, None)

            # keep-warm matmuls spread through the phase-0 window (PE is
            # otherwise idle there and the HAM clock gate would re-throttle;
            # stamps are in scheduler sim-time which runs a few us ahead of
            # the hardware clock)
            for i in range(22):
                with tc.tile_wait_until(0.006 + 0.0005 * i):
                    nc.tensor.matmul(ps_warm, warm_src, warm_src,
                                     start=True, stop=True)

            # ---- phase 1: stream x tiles in groups of 4 ----
            # The whole streaming pipeline is time-gated past the Newton-Schulz
            # window: otherwise the static scheduler packs transpose bursts
            # into every NS dependency gap and stretches the phase-0 latency
            # chain (which gates mm2 and all output DMA) by 2x.
            gate = tc.tile_wait_until(0.014)
            gate.__enter__()
            for g in range(NTILES // GT):
                # xt4 layout [128, (c t n)]: chunk c of all GT tiles adjacent so
                # mm1's rhs for chunk c is the contiguous slice [:, 512c:512c+512].
                # bf16 staging (converted in the ACT copy): only the
                # ~17%-magnitude correction term sees the rounding.
                xt4 = xts.tile([128, GT * 512], bf16, tag="xt4")
                xt4_v = xt4[:, :].rearrange("p (c t n) -> p c t n", c=4, t=GT)
                for t in range(GT):
                    xi = xi2s[2 * g + t // 2][:, 512 * (t % 2) : 512 * (t % 2) + 512]
                    ps_xt = ps_str.tile([128, 512], dt, tag="ps_xt")
                    if t % 2 == 0:
                        # keep-warm filler: transpose-mode doesn't register as
                        # PE activity for the HAM clock gate
                        nc.tensor.matmul(ps_warm, warm_src, warm_src,
                                         start=True, stop=True)
                    for c in range(4):
                        nc.tensor.transpose(
                            ps_xt[:, 128 * c : 128 * (c + 1)],
                            xi[:, 128 * c : 128 * (c + 1)],
                            ident,
                        )
                    nc.scalar.copy(
                        xt4_v[:, :, t, :],
                        ps_xt[:, :].rearrange("p (c n) -> p c n", c=4),
                    )
                ps_u4 = ps_u_pool.tile([64, 512], dt, tag="ps_u2")
                for c in range(4):
                    nc.tensor.matmul(
                        ps_u4,
                        wm[:, 64 * c : 64 * (c + 1)],
                        xt4[:, 512 * c : 512 * (c + 1)],
                        start=(c == 0),
                        stop=(c == 3),
                    )
                u4 = us.tile([64, 512], bf16, tag="u2")
                nc.vector.tensor_copy(u4, ps_u4)
                for half in range(2):
                    ob = outs.tile([128, 1024], bf16, tag="ob")
                    for j in range(2):
                        t = 2 * half + j
                        xi = xi2s[2 * g + t // 2][:, 512 * (t % 2) : 512 * (t % 2) + 512]
                        ps_o = ps_o_pool.tile([128, 512], dt, tag="ps_o")
                        nc.tensor.matmul(
                            ps_o,
                            u4[:, 128 * t : 128 * (t + 1)],
                            ztm,
                            start=True,
                            stop=True,
                        )
                        nc.vector.tensor_add(ob[:, 512 * j : 512 * (j + 1)], xi, ps_o)
                    eng = nc.sync if half == 0 else nc.gpsimd
                    eng.dma_start(
                        o_pair[2 * g + half],
                        ob[:, :].rearrange("p (j f) -> p j f", j=2),
                    )
            gate.__exit__(None, None, None)

    return nc


def make_setup(coeff_b, gate_b, coeff_l_b, gate_l_b, comm_b,
               left, right, left_local, right_local):
    """Pack all small inputs for one batch item into one [128, 708] tensor.
    Pure marshalling: transposes/replication of raw inputs plus constants."""
    f32 = np.float32
    s = np.zeros((128, SETUP_COLS), f32)
    s[:, 0:512] = np.concatenate(
        [right.T, left.T, right_local.T, left_local.T,
         left.T, right.T, left_local.T, right_local.T], axis=0
    )
    s[:, _C_IDENT:_C_IDENT + 128] = np.eye(128, dtype=f32)
    s[0:32, _C_E0 + 32:_C_E0 + 64] = -1.0 / 24.0
    s[32:64, _C_E0:_C_E0 + 32] = 1.0 / 24.0
    ones16 = np.ones(16, f32)
    s[:, _C_BASE] = np.concatenate(
        [ones16, coeff_b, ones16, coeff_l_b, coeff_b, ones16, coeff_l_b, ones16]
    )
    s[:, _C_GATE] = np.concatenate(
        [np.ones(64, f32), np.full(32, gate_b, f32), np.full(32, gate_l_b, f32)]
    )
    s[:, _C_SIGN] = np.concatenate(
        [np.ones(80, f32), -np.ones(16, f32), np.ones(16, f32), -np.ones(16, f32)]
    )
    s[0:64, _C_CVEC] = comm_b
    s[0:64, _C_II:_C_II + 64] = np.eye(64, dtype=f32)
    s[0:64, _C_II + 64:_C_II + 128] = np.eye(64, dtype=f32)
    return s


def make_in_maps(x, coeff, gate, coeff_local, gate_local, comm_scale,
                 left, right, left_local, right_local):
    in_maps = []
    for b in range(x.shape[0]):
        in_maps.append({
            "x": np.ascontiguousarray(x[b]).astype(np.float32),
            "setup": make_setup(coeff[b], gate[b], coeff_local[b], gate_local[b],
                                comm_scale[b], left, right, left_local, right_local),
        })
    return in_maps


def kernel(x, coeff, gate, coeff_local, gate_local, comm_scale,
           left, right, left_local, right_local, _trace=False):
    if "nc" not in _CACHE:
        nc = build_bass()
        nc.finalize()  # Bacc.finalize: compile passes + freeze
        _CACHE["nc"] = nc
    nc = _CACHE["nc"]
    in_maps = make_in_maps(x, coeff, gate, coeff_local, gate_local, comm_scale,
                           left, right, left_local, right_local)
    res = run_bass_kernel_spmd(nc, in_maps, core_ids=list(range(8)), trace=_trace)
    out = np.stack([r["out"] for r in res.results], axis=0)
    if _trace:
        _CACHE["last_results"] = res
    return out.astype(x.dtype)


# revision 44
# speedup vs baseline: 1.0746x; 1.0539x over previous
"""Trainium2 Bass kernel for nn_LowRankOrthogonalMixer (B=8, N=4096, F=512, R=16).

Math: the reference builds per-batch skew matrices G = gate*(A - A^T) with
A = (left*coeff) @ right^T, combines them into
Omega = 0.5*(G+L) + comm/12*(LG-GL), applies the Cayley transform
T = (I-0.5*Omega)^{-1}(I+0.5*Omega), and mixes: out = x @ T.

Key structure exploited: Omega = P @ S @ Q^T with P,Q in [F,64] (rank<=64:
column/row spaces of both skews and their commutator live in
span[left,right,left_local,right_local]). With W = P*(0.5*S) (so
0.5*Omega = W Q^T) and C = I64 - Q^T W, Woodbury collapses the Cayley
transform EXACTLY to
    T = I + 2 W C^{-1} Q^T   =>   out = x + (x @ W) @ (2 C^{-1}) @ Q^T.
C^{-1} comes from 7 Newton-Schulz iterations (V0 = 0.22*C^T, near-optimal
for the measured sigma(C) range [0.55, 2.9]).

Schedule (DMA-roofline oriented; per NeuronCore, data-parallel over B):
- the output is written to HBM as bf16 (quantization ~2e-3 relative, well
  inside the 2e-2 gate) and cast back to fp32 on the host: HBM traffic is
  8 MB x-in + 4 MB out instead of 8+8 (roofline ~35us wire time).
- DMA moves in 2-tile (512 KB in / 256 KB out) units: dma_start costs
  ~650 ns of issuing-engine time regardless of size. All 16 x-in DMAs are
  issued up-front from Sync into dedicated SBUF (no recycling -> no
  anti-deps -> the queue streams 8 MB at line rate); out DMAs alternate
  between Sync and GpSimd.
- phase 0 (C, Newton-Schulz, ZT = 2 C^{-1} Q^T) is emitted at scheduler
  high priority with its latency chain on an otherwise-EMPTY DVE: the
  streaming copies live on ACT, and the DVE residual adds are all gated on
  ztm anyway, so nothing queues ahead of the chain hops in the strict
  in-order DVE FIFO. ztm lands ~15-20us instead of ~37 (the v1 gating).
- phase 1 streams x in 128-row tiles, groups of 4: 4x 128x128 fp32
  PE-transposes per tile, ACT copies the PSUM result to a bf16 xt4
  staging tile, mm1 = W^T x^T in bf16 over 4 F-chunks at N=512,
  mm2 = u @ ZT at N=512 in f32r, DVE adds the fp32 residual from PSUM and
  writes bf16 output pairs which DMA straight out.
- keep-warm: transpose-mode doesn't register as PE activity for the HAM
  clock gate, so a dummy bf16 matmul is sprinkled every other tile
  through the front (plus a ~2.6us warm-up burst at the start).

Sharding: data-parallel over batch B=8 -> one batch item per NeuronCore.
"""

import numpy as np

import concourse.bass as bass
import concourse.bacc as bacc
import concourse.tile as tile
from concourse import mybir
from concourse.bass_utils import run_bass_kernel_spmd

B, N, F, R = 8, 4096, 512, 16
NTILES = N // 128
GT = 4                      # tiles per mm1 group
NPAIR = NTILES // 2         # 2-tile DMA units
ALPHA = 0.22  # Newton-Schulz init scale: V0 = ALPHA * C^T. Near-optimal for the
# measured sigma(C) range [0.55, 2.9]: |1 - a*s^2| <= max(1-0.22*0.30, 0.22*8.41-1)
# = 0.934 -> 7 iterations reach ~1.5e-4; divergence guard a*s_max^2 = 1.85 < 2.
NS_ITERS = 7
WARMUP_MMS = 24

# packed setup tensor column layout
_C_SMALLS = 0        # [128, 512]: rows 0:64 Q^T sources, 64:128 P^T sources
_C_IDENT = 512       # [128, 128] identity
_C_E0 = 640          # [64, 64] +-1/24 commutator mask (rows 64:128 zero)
_C_BASE = 704        # qp_base column
_C_GATE = 705        # qp_gate column
_C_SIGN = 706        # qp_sign column
_C_CVEC = 707        # comm_scale broadcast column (rows 0:64)
_C_II = 708          # [64, 128] = [I64 | I64] (rows 64:128 zero)
SETUP_COLS = 836

_CACHE = {}


def build_bass():
    # Bacc (not plain Bass): its compile() runs move_matmul_waits_to_ldweights
    # + generate_event_semaphores, required because TRN2 instructions support
    # at most one semaphore wait each.
    nc = bacc.Bacc(trn_type="TRN2", target_bir_lowering=False)
    dt = mybir.dt.float32
    bf16 = mybir.dt.bfloat16
    f32r = mybir.dt.float32r

    x_d = nc.dram_tensor("x", [N, F], dt, kind="ExternalInput")
    setup_d = nc.dram_tensor("setup", [128, SETUP_COLS], dt, kind="ExternalInput")
    out_d = nc.dram_tensor("out", [N, F], bf16, kind="ExternalOutput")

    with tile.TileContext(nc) as tc:
        with (
            tc.tile_pool(name="const", bufs=1) as const,
            tc.tile_pool(name="small", bufs=2) as small,
            # all 16 x-in DMAs land in distinct bufs: no anti-deps, the DMA
            # queue streams the full 8 MB back-to-back at line rate
            tc.tile_pool(name="xs", bufs=NPAIR) as xs,
            # shallow xts/us: caps how far the transpose/mm1 front can run
            # ahead of mm2 — front work packed into the Newton-Schulz window
            # stretches the phase-0 latency chain that gates all output
            tc.tile_pool(name="xts", bufs=2) as xts,
            tc.tile_pool(name="us", bufs=2) as us,
            tc.tile_pool(name="outs", bufs=6) as outs,
            tc.tile_pool(name="ps_sm", bufs=2, space="PSUM") as ps_sm,
            tc.tile_pool(name="ps_str", bufs=2, space="PSUM") as ps_str,
            tc.tile_pool(name="ps_u", bufs=1, space="PSUM") as ps_u_pool,
            tc.tile_pool(name="ps_o", bufs=2, space="PSUM") as ps_o_pool,
            tc.tile_pool(name="ps_fill", bufs=1, space="PSUM") as ps_fill_pool,
        ):
            ps_once = ps_sm
            # PE warm-up: ~2.6us of dummy matmuls bridge the gap until the
            # first x tiles arrive (~10us: ring-init + first 512KB DMA) and
            # open the HAM clock gate (K=8/8, 2.4 GHz) before real work.
            warm_src = const.tile([128, 128], bf16)
            nc.vector.memset(warm_src, 0.0)
            ps_warm = ps_fill_pool.tile([128, 128], dt)
            for _ in range(WARMUP_MMS):
                nc.tensor.matmul(ps_warm, warm_src, warm_src,
                                 start=True, stop=True)

            # ---- setup DMAs first on Sync, then ALL x-in DMAs up-front ----
            setup = const.tile([128, SETUP_COLS], dt)
            nc.sync.dma_start(setup, setup_d[:, :])
            setup_p = const.tile([64, SETUP_COLS], dt)
            nc.sync.dma_start(setup_p, setup_d[64:128, :])

            # x as 2-tile units: xi2[k][:, 512*j:512*(j+1)] = x rows
            # 256k+128j .. 256k+128j+128
            x_pair = x_d[:, :].rearrange("(k j p) f -> k p j f", j=2, p=128)
            o_pair = out_d[:, :].rearrange("(k j p) f -> k p j f", j=2, p=128)
            xi2s = []
            for k in range(NPAIR):
                xi2 = xs.tile([128, 1024], dt, tag="xi2")
                nc.sync.dma_start(
                    xi2[:, :].rearrange("p (j f) -> p j f", j=2), x_pair[k]
                )
                xi2s.append(xi2)

            # ---- phase 0 (high priority): W, Q^T, C, C^{-1}, ZT = 2 C^{-1} Q^T ----
            # latency chain on DVE (empty until ztm exists: the residual adds
            # are gated on it); one-off big copies on ACT.
            hp = tc.high_priority()
            hp.__enter__()
            smalls = setup[:, _C_SMALLS:_C_SMALLS + 512]
            ident = setup[:, _C_IDENT:_C_IDENT + 128]
            i64 = setup[0:64, _C_IDENT:_C_IDENT + 64]
            e0 = setup[0:64, _C_E0:_C_E0 + 64]
            base_v = setup[:, _C_BASE:_C_BASE + 1]
            gate_v = setup[:, _C_GATE:_C_GATE + 1]
            sign_v = setup[:, _C_SIGN:_C_SIGN + 1]
            cv = setup[0:64, _C_CVEC:_C_CVEC + 1]

            # qp rows 0:64 = Q^T, rows 64:128 = P^T
            scale = small.tile([128, 1], dt, tag="scale")
            nc.vector.tensor_mul(scale, base_v, gate_v)
            scale2 = small.tile([128, 1], dt, tag="scale2")
            nc.vector.tensor_mul(scale2, scale, sign_v)
            qp = const.tile([128, F], dt)
            nc.vector.tensor_scalar_mul(qp, in0=smalls, scalar1=scale2)
            qt_ap = qp[0:64, :]
            # P^T at base partition 0 for the W^T matmul
            scale_p = small.tile([64, 1], dt, tag="scale_p")
            nc.vector.tensor_mul(
                scale_p,
                setup_p[:, _C_BASE:_C_BASE + 1],
                setup_p[:, _C_GATE:_C_GATE + 1],
            )
            scale_p2 = small.tile([64, 1], dt, tag="scale_p2")
            nc.vector.tensor_mul(scale_p2, scale_p, setup_p[:, _C_SIGN:_C_SIGN + 1])
            pt0 = const.tile([64, F], f32r)
            nc.vector.tensor_scalar_mul(
                pt0, in0=setup_p[:, _C_SMALLS:_C_SMALLS + 512], scalar1=scale_p2
            )

            # naturals: qpn block c (cols 128c..128c+128) = (qp[:,128c:128c+128])^T
            ps_qpn = ps_once.tile([128, 512], dt, tag="ns_ps")
            for c in range(4):
                nc.tensor.transpose(
                    ps_qpn[:, 128 * c : 128 * (c + 1)],
                    qp[:, 128 * c : 128 * (c + 1)],
                    ident,
                )
            qpn = const.tile([128, 512], f32r)
            nc.scalar.copy(qpn, ps_qpn)

            # G1 = Q^T P and G1T = P^T Q side-by-side in ONE PSUM bank so a
            # single DVE cast extracts both (fewer chain hops)
            ps_gg = ps_sm.tile([64, 128], dt, tag="ns_ps")
            for c in range(4):
                qch = qpn[:, 128 * c : 128 * c + 64]
                pch = qpn[:, 128 * c + 64 : 128 * (c + 1)]
                nc.tensor.matmul(ps_gg[:, 0:64], qch, pch, start=(c == 0), stop=(c == 3))
            for c in range(4):
                qch = qpn[:, 128 * c : 128 * c + 64]
                pch = qpn[:, 128 * c + 64 : 128 * (c + 1)]
                nc.tensor.matmul(ps_gg[:, 64:128], pch, qch, start=(c == 0), stop=(c == 3))
            gg = small.tile([64, 128], f32r, tag="gg")
            nc.vector.tensor_copy(gg, ps_gg)
            g1 = gg[:, 0:64]
            g1t = gg[:, 64:128]

            # S_half = 0.25*I + comm * (e0 ⊙ G1)   (e0 carries the ±1/24 pattern)
            e0c = small.tile([64, 64], dt, tag="e0c")
            nc.vector.tensor_scalar_mul(e0c, in0=e0, scalar1=cv)
            s_half = small.tile([64, 64], f32r, tag="s_half")
            nc.vector.tensor_mul(s_half, e0c, g1)
            i4 = small.tile([64, 64], dt, tag="i4")
            nc.scalar.mul(i4, i64, 0.25)
            nc.vector.tensor_add(s_half, s_half, i4)

            # cc = [C | C^T] = [I|I] - [G1 S_half | (G1 S_half)^T], one bank +
            # one DVE sub
            ii = setup[0:64, _C_II:_C_II + 128]
            ps_cc = ps_sm.tile([64, 128], dt, tag="ns_ps")
            nc.tensor.matmul(ps_cc[:, 0:64], g1t, s_half, start=True, stop=True)
            nc.tensor.matmul(ps_cc[:, 64:128], s_half, g1t, start=True, stop=True)
            cc = small.tile([64, 128], f32r, tag="cc")
            nc.vector.tensor_sub(cc, ii, ps_cc)
            cmat = cc[:, 0:64]     # C
            ctm = cc[:, 64:128]    # C^T

            # W^T = S_half^T @ P^T  [64, F]; then W natural in 4 chunks [128, 64]
            # (off the NS critical path; ACT does the copies; W in bf16 for mm1)
            ps_wt = ps_once.tile([128, 512], dt, tag="ns_ps")
            nc.tensor.matmul(ps_wt[0:64, :], s_half, pt0, start=True, stop=True)
            wtm = const.tile([64, 512], dt)
            nc.scalar.copy(wtm, ps_wt[0:64, :])
            ps_w = ps_once.tile([128, 512], dt, tag="ns_ps")
            for c in range(4):
                nc.tensor.transpose(
                    ps_w[:, 64 * c : 64 * (c + 1)],
                    wtm[:, 128 * c : 128 * (c + 1)],
                    i64,
                )
            wm = const.tile([128, 256], bf16)
            nc.scalar.copy(wm, ps_w[:, 0:256])

            # Newton-Schulz for V = C^{-1}. vv = [V^T | V]: both iterates live
            # side-by-side so each iteration is mm -> sub -> 2x mm (one bank)
            # -> ONE cast. (V error after 7 iters ~1.5e-4, far below the bf16
            # rounding of ztm, so everything stays f32r.)
            i2 = small.tile([64, 64], f32r, tag="i2")
            nc.scalar.mul(i2, i64, 2.0)
            vv = small.tile([64, 128], f32r, tag="vv")
            nc.vector.tensor_scalar_mul(vv, in0=cc, scalar1=ALPHA)  # [a*C | a*C^T] = [vt0 | v0]
            for it in range(NS_ITERS):
                ps_t1 = ps_sm.tile([64, 128], dt, tag="ns_ps")
                nc.tensor.matmul(ps_t1[:, 0:64], ctm, vv[:, 64:128],
                                 start=True, stop=True)              # C V
                t2 = small.tile([64, 64], f32r, tag="t2")
                nc.vector.tensor_sub(t2, i2, ps_t1[:, 0:64])         # 2I - CV
                ps_vv = ps_sm.tile([64, 128], dt, tag="ns_ps")
                nc.tensor.matmul(ps_vv[:, 0:64], t2, vv[:, 0:64],
                                 start=True, stop=True)              # t2^T V^T = vt'
                nc.tensor.matmul(ps_vv[:, 64:128], vv[:, 0:64], t2,
                                 start=True, stop=True)              # V t2 = v'
                vv = small.tile([64, 128], f32r, tag="vv")
                nc.vector.tensor_copy(vv, ps_vv)

            # ZT = 2 * V @ Q^T  [64, F] (bf16 for the FWL-accelerated mm2)
            ps_zt = ps_once.tile([128, 512], dt, tag="ns_ps")
            nc.tensor.matmul(ps_zt[0:64, :], vv[:, 0:64].bitcast(dt), qt_ap,
                             start=True, stop=True)
            ztm = const.tile([64, 512], bf16)
            nc.vector.tensor_scalar_mul(ztm, in0=ps_zt[0:64, :], scalar1=2.0)
            hp.__exit__(None, None, None)



            # ---- phase 1: stream x tiles in groups of 4 ----
            for g in range(NTILES // GT):
                # xt4 layout [128, (c t n)]: chunk c of all GT tiles adjacent so
                # mm1's rhs for chunk c is the contiguous slice [:, 512c:512c+512].
                # bf16 staging (converted in the ACT copy): only the
                # ~17%-magnitude correction term sees the rounding.
                xt4 = xts.tile([128, GT * 512], bf16, tag="xt4")
                xt4_v = xt4[:, :].rearrange("p (c t n) -> p c t n", c=4, t=GT)
                for t in range(GT):
                    xi = xi2s[2 * g + t // 2][:, 512 * (t % 2) : 512 * (t % 2) + 512]
                    ps_xt = ps_str.tile([128, 512], dt, tag="ps_xt")
                    if t % 2 == 0:
                        # keep-warm filler: transpose-mode doesn't register as
                        # PE activity for the HAM clock gate
                        nc.tensor.matmul(ps_warm, warm_src, warm_src,
                                         start=True, stop=True)
                    for c in range(4):
                        nc.tensor.transpose(
                            ps_xt[:, 128 * c : 128 * (c + 1)],
                            xi[:, 128 * c : 128 * (c + 1)],
                            ident,
                        )
                    nc.scalar.copy(
                        xt4_v[:, :, t, :],
                        ps_xt[:, :].rearrange("p (c n) -> p c n", c=4),
                    )
                ps_u4 = ps_u_pool.tile([64, 512], dt, tag="ps_u2")
                for c in range(4):
                    nc.tensor.matmul(
                        ps_u4,
                        wm[:, 64 * c : 64 * (c + 1)],
                        xt4[:, 512 * c : 512 * (c + 1)],
                        start=(c == 0),
                        stop=(c == 3),
                    )
                u4 = us.tile([64, 512], bf16, tag="u2")
                nc.vector.tensor_copy(u4, ps_u4)
                for half in range(2):
                    ob = outs.tile([128, 1024], bf16, tag="ob")
                    for j in range(2):
                        t = 2 * half + j
                        xi = xi2s[2 * g + t // 2][:, 512 * (t % 2) : 512 * (t % 2) + 512]
                        ps_o = ps_o_pool.tile([128, 512], dt, tag="ps_o")
                        nc.tensor.matmul(
                            ps_o,
                            u4[:, 128 * t : 128 * (t + 1)],
                            ztm,
                            start=True,
                            stop=True,
                        )
                        nc.vector.tensor_add(ob[:, 512 * j : 512 * (j + 1)], xi, ps_o)
                    eng = nc.sync if half == 0 else nc.gpsimd
                    eng.dma_start(
                        o_pair[2 * g + half],
                        ob[:, :].rearrange("p (j f) -> p j f", j=2),
                    )

    return nc


def make_setup(coeff_b, gate_b, coeff_l_b, gate_l_b, comm_b,
               left, right, left_local, right_local):
    """Pack all small inputs for one batch item into one [128, 708] tensor.
    Pure marshalling: transposes/replication of raw inputs plus constants."""
    f32 = np.float32
    s = np.zeros((128, SETUP_COLS), f32)
    s[:, 0:512] = np.concatenate(
        [right.T, left.T, right_local.T, left_local.T,
         left.T, right.T, left_local.T, right_local.T], axis=0
    )
    s[:, _C_IDENT:_C_IDENT + 128] = np.eye(128, dtype=f32)
    s[0:32, _C_E0 + 32:_C_E0 + 64] = -1.0 / 24.0
    s[32:64, _C_E0:_C_E0 + 32] = 1.0 / 24.0
    ones16 = np.ones(16, f32)
    s[:, _C_BASE] = np.concatenate(
        [ones16, coeff_b, ones16, coeff_l_b, coeff_b, ones16, coeff_l_b, ones16]
    )
    s[:, _C_GATE] = np.concatenate(
        [np.ones(64, f32), np.full(32, gate_b, f32), np.full(32, gate_l_b, f32)]
    )
    s[:, _C_SIGN] = np.concatenate(
        [np.ones(80, f32), -np.ones(16, f32), np.ones(16, f32), -np.ones(16, f32)]
    )
    s[0:64, _C_CVEC] = comm_b
    s[0:64, _C_II:_C_II + 64] = np.eye(64, dtype=f32)
    s[0:64, _C_II + 64:_C_II + 128] = np.eye(64, dtype=f32)
    return s


def make_in_maps(x, coeff, gate, coeff_local, gate_local, comm_scale,
                 left, right, left_local, right_local):
    in_maps = []
    for b in range(x.shape[0]):
        in_maps.append({
            "x": np.ascontiguousarray(x[b]).astype(np.float32),
            "setup": make_setup(coeff[b], gate[b], coeff_local[b], gate_local[b],
                                comm_scale[b], left, right, left_local, right_local),
        })
    return in_maps


def kernel(x, coeff, gate, coeff_local, gate_local, comm_scale,
           left, right, left_local, right_local, _trace=False):
    if "nc" not in _CACHE:
        nc = build_bass()
        nc.finalize()  # Bacc.finalize: compile passes + freeze
        _CACHE["nc"] = nc
    nc = _CACHE["nc"]
    in_maps = make_in_maps(x, coeff, gate, coeff_local, gate_local, comm_scale,
                           left, right, left_local, right_local)
    res = run_bass_kernel_spmd(nc, in_maps, core_ids=list(range(8)), trace=_trace)
    out = np.stack([r["out"] for r in res.results], axis=0)
    if _trace:
        _CACHE["last_results"] = res
    return out.astype(x.dtype)
